# revision 33
# baseline (speedup 1.0000x reference)
"""AffinityPropagate Trainium2 kernel.

Reference computation (per batch element):
    k_d = softmax(guided_d, axis=channel)          d = 1,2,3 (dilations)
    repeat 8 times:
        o_d = sum_ch k_d[ch] * shift(x, offset(d, ch))
        x   = o_1*fuse[0] + o_2*fuse[1] + o_3*fuse[2]

Strategy: pure data parallel over the batch (8 batches -> 8 NeuronCores).
Per core, the three 9-tap dilated kernels are pre-fused with the fuse
weights into 25 distinct-offset weight fields (the three (0,0) taps
share one field) stored in fp16 in SBUF.  x is kept in a halo layout:
partition p owns image rows [4p, 4p+4), stored with 3 halo rows on each
side and 4 zero border columns on each side ([120, 10, 648] fp16).

24 of the weight fields are packed into 9 row-group tiles (per
dilation, per row offset: the 2-3 taps whose column offsets form an
arithmetic progression).  Each iteration then needs only 10 DVE
tensor_mul instructions: the x side of each group reads 2-3
overlapping shifted windows through a 4-dim access pattern
([P][tap, stride d][row][col]), so the per-instruction overhead is
paid ~10x instead of 25x.  The fused (0,0) center field is the 25th:
it multiplies the PREVIOUS iteration's PSUM directly (f32, unshifted,
so no border-garbage issue), overlapping ScalarE's PSUM->x
evacuation at the iteration boundary.  TensorE accumulates the 25
product fields into PSUM in fp32 via identity-stationary matmuls;
halo rows are rebuilt by TensorE with shift-by-one-partition matmuls
through single-bank PSUM chunks under high_priority (PE runs them
before the next iteration's accumulation, limiting LDWEIGHTS thrash
between ident and the shift matrices).

The guided tensors stream across three DMA rings (sync HWDGE, scalar
HWDGE, gpsimd SWDGE), rotated per dilation so dilation 3's
last-arriving channels ride the faster HWDGE rings.  Iteration-1 taps
of dilations 1-2 run under the stream with normalized weights;
dilation 3 - whose weights land last - instead accumulates raw
exp-weighted taps and its channel sum on DVE per-channel as they
arrive (all hidden under the DMA), then applies the per-pixel
fuse3/sum3 scale once at the end, collapsing the exposed
post-stream tail from ~44us to ~15us.  The final iteration's PSUM is
evacuated and DMAed to DRAM in halves across both HWDGE rings.
"""

import numpy as np

import concourse.bacc as bacc
import concourse.bass as bass
import concourse.mybir as mybir
from concourse.bass_utils import run_bass_kernel_spmd
from concourse.masks import make_identity
from concourse.tile import TileContext

H, W = 480, 640
P = 120          # partitions used (each owns R rows)
R = 4            # rows per partition
HALO = 3         # halo rows each side
CB = 4           # border cols each side (4 keeps packed reads 4B aligned)
ROWB = R + 2 * HALO          # 10 buffer rows per partition
COLB = W + 2 * CB            # 648 buffer cols
NFLAT = ROWB * COLB
RW = R * W
PROP_TIME = 8
NCORES = 8

F16 = mybir.dt.float16
F32 = mybir.dt.float32


def _make_groups():
    """9 tap groups: per (dilation, row offset), the taps whose column
    offsets form an arithmetic progression (so one DVE instruction can
    read all of them through a strided 4-dim AP).  The fused (0,0)
    center tap is a standalone 25th field: in iterations it reads the
    previous PSUM directly (f32, unshifted), so it can run while
    ScalarE is still evacuating that PSUM to the fp16 x buffer."""
    groups = []   # (d, dh, [dw...])
    slot = {}     # (dh, dw) -> (group idx, slot idx); (0,0) handled apart
    for d in (1, 2, 3):
        for r in (-1, 0, 1):
            dh = r * d
            if r == 0:
                dws = [-1, 1] if d == 1 else [-d, d]
            else:
                dws = [-d, 0, d]
            gi = len(groups)
            groups.append((d, dh, dws))
            for si, dw in enumerate(dws):
                slot[(dh, dw)] = (gi, si)
    return groups, slot


GROUPS, SLOT = _make_groups()
assert sum(len(g[2]) for g in GROUPS) == 24
# halo-independent (dh == 0) groups first
_MID = [gi for gi, g in enumerate(GROUPS) if g[1] == 0]
EMIT_ORDER = [_MID[1], _MID[0], _MID[2]] + \
             [gi for gi, g in enumerate(GROUPS) if g[1] != 0]


def build_nc():
    nc = bacc.Bacc("TRN2", target_bir_lowering=False, debug=False)

    g_dram = [
        nc.dram_tensor(name, [9, H, W], F32, kind="ExternalInput")
        for name in ("guided1", "guided2", "guided3")
    ]
    fuse_dram = nc.dram_tensor("fuse", [3, H, W], F32, kind="ExternalInput")
    x_dram = nc.dram_tensor("x", [1, H, W], F32, kind="ExternalInput")
    out_dram = nc.dram_tensor("out", [1, H, W], F32, kind="ExternalOutput")

    # DRAM access patterns: partition p <- rows [4p, 4p+4) (one
    # contiguous 10KB descriptor per partition)
    def rows_ap(t, extra_off=0, cols=RW):
        return bass.AP(t, extra_off, [[RW, P], [1, cols]])

    uid = [0]

    def nm(pfx):
        uid[0] += 1
        return f"{pfx}{uid[0]}"

    with TileContext(nc) as tc:
        with (
            tc.tile_pool(name="const", bufs=1) as constp,
            tc.tile_pool(name="wpool", bufs=1) as wpool,
            tc.tile_pool(name="xmain", bufs=1) as xmain,
            tc.tile_pool(name="psit", bufs=1, space="PSUM") as psi,
        ):
            ident = constp.tile([P, P], F16)
            make_identity(nc, ident)
            # shift-by-one-partition matrices: S_up moves partition p-1's
            # data to p (top halo), S_dn the reverse
            S_up = constp.tile([P, P], F16, tag="sup")
            S_dn = constp.tile([P, P], F16, tag="sdn")
            for tile_, base in ((S_up, 1), (S_dn, -1)):
                nc.gpsimd.memset(tile_, 0.0)
                nc.gpsimd.affine_select(
                    out=tile_, in_=tile_,
                    compare_op=mybir.AluOpType.not_equal,
                    fill=1.0, base=base, pattern=[[-1, P]],
                    channel_multiplier=1,
                )

            # weight group tiles (25 fields total, 5KB/partition each)
            wg = [wpool.tile([P, len(dws), R, W], F16, tag=f"wg{gi}",
                             name=f"wg{gi}")
                  for gi, (d, dh, dws) in enumerate(GROUPS)]
            wc = wpool.tile([P, 1, R, W], F16, tag="wc", name="wc")
            XA = xmain.tile([P, ROWB, COLB], F16, tag="XA")
            XB = xmain.tile([P, ROWB, COLB], F16, tag="XB")

            def slot_view(dh, dw):
                if (dh, dw) == (0, 0):
                    return wc[:, 0:1, :, :]
                gi, si = SLOT[(dh, dw)]
                return wg[gi][:, si:si + 1, :, :]       # [P, 1, R, W]

            mm_n = [0]

            def acc(ps, mflat, base, stop):
                first = mm_n[0] == 0
                for k in range(5):
                    nc.tensor.matmul(
                        out=ps[:, k * 512:(k + 1) * 512],
                        lhsT=ident,
                        rhs=mflat[:, base + k * 512:base + (k + 1) * 512],
                        start=first, stop=stop,
                    )
                    mm_n[0] += 1

            def emit_taps_single(taps, Xc, ps, pool, stop_at_end):
                """Per-tap emission (setup iteration 1, hidden under DMA)."""
                for ti, (dh, dw) in enumerate(taps):
                    m = pool.tile([P, 1, R, W], F16, tag="m0",
                                  name=nm("m0_"))
                    win = Xc[:, HALO + dh:HALO + dh + R,
                             CB + dw:CB + dw + W]
                    nc.vector.tensor_mul(
                        out=m, in0=slot_view(dh, dw), in1=win.unsqueeze(1),
                    )
                    mf = m.rearrange("p s a b -> p (s a b)")
                    acc(ps, mf, 0, stop_at_end and ti == len(taps) - 1)

            def emit_taps_batched(Xc, ps, pool3, stop_at_end,
                                  pending=None):
                """9 batched tap groups.  `pending` is the center tap's
                product: its accumulation is deferred until after the first
                group's (so the center DVE multiply is a pure READER of the
                previous PSUM and never inherits the write-after-read wait
                on the evacuation)."""
                Xfull = Xc[:, :, :]
                pstr = Xfull.ap[0][0]
                assert Xfull.ap[1][0] == COLB and Xfull.ap[2][0] == 1, \
                    Xfull.ap
                for oi, gi in enumerate(EMIT_ORDER):
                    d, dh, dws = GROUPS[gi]
                    n = len(dws)
                    step = dws[1] - dws[0]
                    m = pool3.tile([P, 3, R, W], F16, tag="m", name=nm("m_"))
                    base_off = Xfull.offset + (HALO + dh) * COLB \
                        + (CB + dws[0])
                    win = bass.AP(
                        Xfull.tensor, base_off,
                        [[pstr, P], [step, n], [COLB, R], [1, W]],
                    )
                    nc.vector.tensor_mul(
                        out=m[:, 0:n], in0=wg[gi], in1=win,
                    )
                    mf = m.rearrange("p s a b -> p (s a b)")
                    last_g = oi == len(EMIT_ORDER) - 1
                    for si in range(n):
                        acc(ps, mf, si * RW,
                            stop_at_end and last_g and si == n - 1)
                    if oi == 0 and pending is not None:
                        acc(ps, pending.rearrange("p s a b -> p (s a b)"),
                            0, False)

            HB3 = HALO * COLB      # 1944, halo bytes span per side

            def emit_tail(Xn, ps, last_iter, shiftp):
                """Evacuate PSUM, rebuild halo rows via PE partition
                shifts through single-bank PSUM chunks."""
                if last_iter:
                    return
                # the evac must NOT be high-priority: it shares the previous
                # PSUM with the center tap emitted just before it, and reader
                # serialization follows scheduler order -- the center tap has
                # to come first so it overlaps the evac
                nc.scalar.copy(
                    out=Xn[:, HALO:HALO + R, CB:CB + W],
                    in_=ps.rearrange("p (a b) -> p a b", a=R),
                )
                with tc.high_priority():
                    Xn_f = Xn.rearrange("p a b -> p (a b)")
                    for S, src0, dst0 in (
                        (S_up, (HALO + 1) * COLB, 0),
                        (S_dn, HALO * COLB, (R + HALO) * COLB),
                    ):
                        for c0 in range(0, HB3, 512):
                            ln = min(512, HB3 - c0)
                            psh = shiftp.tile([P, 512], F32, tag="sh",
                                              name=nm("sh_"))
                            nc.tensor.matmul(
                                out=psh[:, 0:ln], lhsT=S,
                                rhs=Xn_f[:, src0 + c0:src0 + c0 + ln],
                                start=True, stop=True,
                            )
                            nc.scalar.copy(
                                out=Xn_f[:, dst0 + c0:dst0 + c0 + ln],
                                in_=psh[:, 0:ln],
                            )

            # ---------------- setup + iteration 1 ----------------
            # Guided tensors stream in per dilation across 3 DMA rings;
            # iteration-1 taps of each dilation run as soon as that
            # dilation's weights are ready, hiding compute under the DMA.
            ps0 = psi.tile([P, RW], F32, tag="ps", name="ps_it0")
            with (
                tc.tile_pool(name="setup", bufs=2) as sp,
                tc.tile_pool(name="fusep", bufs=1) as fusep,
                tc.tile_pool(name="m0pool", bufs=1) as m0pool,
                tc.tile_pool(name="psst", bufs=1, space="PSUM") as psp,
            ):
                nc.vector.memset(XA, 0.0)
                nc.vector.memset(XB, 0.0)
                # three balanced DMA rings; rotated per dilation so that
                # dilation 3's last channels (the only exposed stragglers)
                # land on the faster HWDGE rings, not gpsimd's SWDGE
                engs = [nc.sync, nc.scalar, nc.gpsimd]

                def ring_of(d_idx, ch):
                    return engs[(ch + d_idx) % 3]

                for d_idx in range(2):
                    d = d_idx + 1

                    def off_of(ch):
                        return ((ch // 3 - 1) * d, (ch % 3 - 1) * d)

                    f16 = fusep.tile([P, R, W], F16, tag="f16",
                                     name=nm("f16_"))
                    e_c = fusep.tile([P, R, W], F16, tag="ec", name=nm("ec_"))
                    for ch in range(9):
                        g = sp.tile([P, RW], F32, tag="g", name=nm("g_"))
                        ring_of(d_idx, ch).dma_start(
                            out=g, in_=rows_ap(g_dram[d_idx], ch * H * W)
                        )
                        if ch == 4 and d_idx > 0:
                            dest = e_c.unsqueeze(1)
                        else:
                            dest = slot_view(*off_of(ch))
                        nc.scalar.activation(
                            out=dest.rearrange("p s a b -> p (s a b)"),
                            in_=g,
                            func=mybir.ActivationFunctionType.Exp,
                        )
                    f32t = sp.tile([P, RW], F32, tag="g", name=nm("f32t_"))
                    nc.scalar.dma_start(
                        out=f32t, in_=rows_ap(fuse_dram, d_idx * H * W)
                    )
                    nc.vector.tensor_copy(
                        out=f16, in_=f32t.rearrange("p (a b) -> p a b", a=R)
                    )
                    if d_idx == 0:
                        # x load rides behind dilation 1's channels --
                        # it's only needed once iteration-1 taps start
                        xs32 = sp.tile([P, RW], F32, tag="g")
                        nc.sync.dma_start(out=xs32, in_=rows_ap(x_dram))
                        nc.vector.tensor_copy(
                            out=XA[:, HALO:HALO + R, CB:CB + W],
                            in_=xs32.rearrange("p (a b) -> p a b", a=R),
                        )
                        XA_f = XA.rearrange("p a b -> p (a b)")
                        nc.sync.dma_start(
                            out=XA_f[1:P, 0:HALO * COLB],
                            in_=XA_f[0:P - 1, R * COLB:(R + HALO) * COLB],
                        )
                        nc.sync.dma_start(
                            out=XA_f[0:P - 1, (R + HALO) * COLB:NFLAT],
                            in_=XA_f[1:P, HALO * COLB:2 * HALO * COLB],
                        )
                    # channel sums + 1/sum in halves (3 PSUM banks)
                    t_ = fusep.tile([P, RW], F16, tag="t", name=nm("t_"))
                    f16f = f16.rearrange("p a b -> p (a b)")
                    HB = RW // 2
                    for h0 in (0, HB):
                        pss = psp.tile([P, HB], F32, tag="pss",
                                       name=nm("pss_"))
                        for ch in range(9):
                            if ch == 4 and d_idx > 0:
                                sf = e_c.rearrange("p a b -> p (a b)")
                            else:
                                sf = slot_view(*off_of(ch)).rearrange(
                                    "p s a b -> p (s a b)")
                            for c0 in range(0, HB, 512):
                                c1 = min(c0 + 512, HB)
                                nc.tensor.matmul(
                                    out=pss[:, c0:c1], lhsT=ident,
                                    rhs=sf[:, h0 + c0:h0 + c1],
                                    start=(ch == 0), stop=(ch == 8),
                                )
                        r = fusep.tile([P, HB], F32, tag="r", name=nm("r_"))
                        nc.vector.reciprocal_approx_fast(out=r, in_=pss)
                        nc.vector.tensor_mul(
                            out=t_[:, h0:h0 + HB], in0=f16f[:, h0:h0 + HB],
                            in1=r,
                        )
                    tv = t_.rearrange("p (a b) -> p a b", a=R)
                    # normalize this dilation's groups in-place
                    for gi in (3 * d_idx, 3 * d_idx + 1, 3 * d_idx + 2):
                        n = len(GROUPS[gi][2])
                        tvb = tv.unsqueeze(1).broadcast_to((P, n, R, W))
                        nc.vector.tensor_mul(out=wg[gi], in0=wg[gi], in1=tvb)
                    if d_idx == 0:
                        # center field: exp(g1_c) -> k1_c * f1
                        nc.vector.tensor_mul(
                            out=wc, in0=wc, in1=tv.unsqueeze(1),
                        )
                    else:
                        cview = slot_view(0, 0)
                        nc.vector.tensor_mul(
                            out=e_c, in0=e_c, in1=tv,
                        )
                        nc.vector.tensor_add(
                            out=cview, in0=cview, in1=e_c.unsqueeze(1),
                        )
                    # iteration-1 taps of this dilation; the shared center
                    # field holds d1+d2's contribution once d2 is folded in,
                    # so the center tap runs at the end of d2 (d3's center
                    # share rides in the raw-tap path below)
                    taps = [off_of(ch) for ch in range(9) if ch != 4]
                    if d_idx == 1:
                        taps.append((0, 0))
                    emit_taps_single(taps, XA, ps0, m0pool, stop_at_end=False)

                # ---- dilation 3: iteration-1 via RAW exp taps ----
                # d3's weights land last, so anything serialized after them
                # is exposed.  Instead of waiting for the softmax
                # normalization, accumulate sum_ch exp(g_ch) * shift(x) in
                # fp16 SBUF and the channel sum on DVE as each channel
                # arrives (all hidden under the DMA stream), then apply the
                # per-pixel fuse3/sum3 scale once at the end.
                f16 = fusep.tile([P, R, W], F16, tag="f16", name=nm("f16_"))
                e_c = fusep.tile([P, R, W], F16, tag="ec", name=nm("ec_"))
                raw = fusep.tile([P, 1, R, W], F16, tag="raw")
                sacc = fusep.tile([P, 1, R, W], F16, tag="sacc")
                f32t = sp.tile([P, RW], F32, tag="g", name=nm("f32t3_"))
                nc.gpsimd.dma_start(
                    out=f32t, in_=rows_ap(fuse_dram, 2 * H * W)
                )
                nc.vector.tensor_copy(
                    out=f16, in_=f32t.rearrange("p (a b) -> p a b", a=R)
                )
                for ch in range(9):
                    g = sp.tile([P, RW], F32, tag="g", name=nm("g_"))
                    ring_of(2, ch).dma_start(
                        out=g, in_=rows_ap(g_dram[2], ch * H * W)
                    )
                    dh, dw = (ch // 3 - 1) * 3, (ch % 3 - 1) * 3
                    if ch == 4:
                        dest = e_c.unsqueeze(1)
                    else:
                        dest = slot_view(dh, dw)
                    nc.scalar.activation(
                        out=dest.rearrange("p s a b -> p (s a b)"), in_=g,
                        func=mybir.ActivationFunctionType.Exp,
                    )
                    win = XA[:, HALO + dh:HALO + dh + R,
                             CB + dw:CB + dw + W].unsqueeze(1)
                    if ch == 0:
                        nc.vector.tensor_mul(out=raw, in0=dest, in1=win)
                        nc.vector.tensor_copy(out=sacc, in_=dest)
                    else:
                        pr = m0pool.tile([P, 1, R, W], F16, tag="m0",
                                         name=nm("pr_"))
                        nc.vector.tensor_mul(out=pr, in0=dest, in1=win)
                        nc.vector.tensor_add(out=raw, in0=raw, in1=pr)
                        nc.vector.tensor_add(out=sacc, in0=sacc, in1=dest)
                # t3 = fuse3 / sum3 (per pixel); the f32 staging borrows a
                # free g-buffer (the channel stream is over), reciprocal in
                # halves to reuse the d1/d2-sized tiles
                s32 = sp.tile([P, RW], F32, tag="g", name=nm("s32_"))
                nc.vector.tensor_copy(
                    out=s32, in_=sacc.rearrange("p s a b -> p (s a b)")
                )
                t_ = fusep.tile([P, RW], F16, tag="t", name=nm("t3_"))
                saccf = s32
                f16f3 = f16.rearrange("p a b -> p (a b)")
                HB = RW // 2
                for h0 in (0, HB):
                    rh = fusep.tile([P, HB], F32, tag="r", name=nm("r3_"))
                    nc.vector.reciprocal_approx_fast(
                        out=rh, in_=saccf[:, h0:h0 + HB]
                    )
                    nc.vector.tensor_mul(
                        out=t_[:, h0:h0 + HB], in0=f16f3[:, h0:h0 + HB],
                        in1=rh,
                    )
                tv = t_.rearrange("p (a b) -> p a b", a=R)
                # iteration-1 d3 contribution (carries the stop flags)
                m3 = m0pool.tile([P, 1, R, W], F16, tag="m0", name=nm("m3_"))
                nc.vector.tensor_mul(out=m3, in0=raw, in1=tv.unsqueeze(1))
                acc(ps0, m3.rearrange("p s a b -> p (s a b)"), 0, True)
                assert mm_n[0] == 90
                # weights for iterations 2..8: fold e_c into the shared
                # center field and normalize d3's groups (these run on DVE
                # concurrently with iteration 1's PSUM evacuation)
                cview = slot_view(0, 0)
                nc.vector.tensor_mul(out=e_c, in0=e_c, in1=tv)
                nc.vector.tensor_add(
                    out=cview, in0=cview, in1=e_c.unsqueeze(1)
                )
                for gi in (6, 7, 8):
                    n = len(GROUPS[gi][2])
                    tvb = tv.unsqueeze(1).broadcast_to((P, n, R, W))
                    nc.vector.tensor_mul(out=wg[gi], in0=wg[gi], in1=tvb)

            # ---------------- iterations 2..8 ----------------
            with tc.tile_pool(name="shp", bufs=3, space="PSUM") as shiftp:
                with (
                    tc.tile_pool(name="mpool", bufs=3) as mpool,
                    tc.tile_pool(name="mcpool", bufs=1) as mcpool,
                ):
                    bufs = [XA, XB]
                    ps = ps0
                    for it in range(1, PROP_TIME):
                        Xc = bufs[it % 2]
                        mm_n[0] = 0
                        new_ps = psi.tile([P, RW], F32, tag="ps",
                                          name=nm("ps_"))
                        # center tap: FIRST reader of the previous PSUM,
                        # emitted before its evacuation so DVE runs it while
                        # ScalarE evacuates (readers are serialized in
                        # emission order)
                        mC = mcpool.tile([P, 1, R, W], F16, tag="mc",
                                         name=nm("mc_"))
                        pswin = ps.rearrange("p (a b) -> p a b",
                                             a=R).unsqueeze(1)
                        nc.vector.tensor_mul(out=mC, in0=wc, in1=pswin)
                        emit_tail(Xc, ps, False, shiftp)
                        emit_taps_batched(Xc, new_ps, mpool,
                                          stop_at_end=True, pending=mC)
                        assert mm_n[0] == 125
                        ps = new_ps

                # final iteration: evacuate PSUM in halves so the DMA of
                # the first half overlaps the copy of the second, and the
                # two DMAs ride different HWDGE rings
                with tc.tile_pool(name="stagep", bufs=1) as stagep:
                    stage = stagep.tile([P, RW], F32)
                    nc.scalar.copy(out=stage[:, 0:RW // 2],
                                   in_=ps[:, 0:RW // 2])
                    nc.sync.dma_start(
                        out=rows_ap(out_dram, 0, cols=RW // 2),
                        in_=stage[:, 0:RW // 2],
                    )
                    nc.scalar.copy(out=stage[:, RW // 2:RW],
                                   in_=ps[:, RW // 2:RW])
                    nc.scalar.dma_start(
                        out=rows_ap(out_dram, RW // 2, cols=RW // 2),
                        in_=stage[:, RW // 2:RW],
                    )

    nc.compile()
    return nc


_NC = None


def _get_nc():
    global _NC
    if _NC is None:
        _NC = build_nc()
    return _NC


def _in_maps(guided1, guided2, guided3, fuse, x):
    maps = []
    for b in range(NCORES):
        maps.append({
            "guided1": np.ascontiguousarray(guided1[b], dtype=np.float32),
            "guided2": np.ascontiguousarray(guided2[b], dtype=np.float32),
            "guided3": np.ascontiguousarray(guided3[b], dtype=np.float32),
            "fuse": np.ascontiguousarray(fuse[b], dtype=np.float32),
            "x": np.ascontiguousarray(x[b], dtype=np.float32),
        })
    return maps


def kernel(guided1, guided2, guided3, fuse, x):
    nc = _get_nc()
    res = run_bass_kernel_spmd(
        nc, _in_maps(guided1, guided2, guided3, fuse, x),
        core_ids=list(range(NCORES)),
    )
    return np.stack([res.results[b]["out"] for b in range(NCORES)], axis=0)


def kernel_profiled(guided1, guided2, guided3, fuse, x, tmpdir=None):
    """Returns (output, BassKernelResults) with trace enabled."""
    nc = _get_nc()
    res = run_bass_kernel_spmd(
        nc, _in_maps(guided1, guided2, guided3, fuse, x),
        core_ids=list(range(NCORES)), trace=True, tmpdir=tmpdir,
    )
    out = np.stack([res.results[b]["out"] for b in range(NCORES)], axis=0)
    return out, res


# revision 34
# speedup vs baseline: 1.1865x; 1.1865x over previous
"""AffinityPropagate Trainium2 kernel.

Reference computation (per batch element):
    k_d = softmax(guided_d, axis=channel)          d = 1,2,3 (dilations)
    repeat 8 times:
        o_d = sum_ch k_d[ch] * shift(x, offset(d, ch))
        x   = o_1*fuse[0] + o_2*fuse[1] + o_3*fuse[2]

Strategy: pure data parallel over the batch (8 batches -> 8 NeuronCores).
Per core, the three 9-tap dilated kernels are pre-fused with the fuse
weights into 25 distinct-offset weight fields (the three (0,0) taps
share one field) stored in fp16 in SBUF.  x is kept in a halo layout:
partition p owns image rows [4p, 4p+4), stored with 3 halo rows on each
side and 4 zero border columns on each side ([120, 10, 648] fp16).

24 of the weight fields are packed into 9 row-group tiles (per
dilation, per row offset: the 2-3 taps whose column offsets form an
arithmetic progression).  Each iteration then needs only 10 DVE
tensor_mul instructions: the x side of each group reads 2-3
overlapping shifted windows through a 4-dim access pattern
([P][tap, stride d][row][col]), so the per-instruction overhead is
paid ~10x instead of 25x.  The fused (0,0) center field is the 25th:
it multiplies the PREVIOUS iteration's PSUM directly (f32, unshifted,
so no border-garbage issue), overlapping ScalarE's PSUM->x
evacuation at the iteration boundary.  TensorE accumulates the 25
product fields into PSUM in fp32 via identity-stationary matmuls;
halo rows are rebuilt by TensorE with shift-by-one-partition matmuls
through single-bank PSUM chunks under high_priority (PE runs them
before the next iteration's accumulation, limiting LDWEIGHTS thrash
between ident and the shift matrices).

The guided tensors stream across three DMA rings (sync HWDGE, scalar
HWDGE, gpsimd SWDGE), rotated per dilation so dilation 3's
last-arriving channels ride the faster HWDGE rings.  Iteration-1 taps
of dilations 1-2 run under the stream with normalized weights;
dilation 3 - whose weights land last - instead accumulates raw
exp-weighted taps and its channel sum on DVE per-channel as they
arrive (all hidden under the DMA), then applies the per-pixel
fuse3/sum3 scale once at the end, collapsing the exposed
post-stream tail from ~44us to ~15us.  The final iteration's PSUM is
evacuated and DMAed to DRAM in halves across both HWDGE rings.
"""

import numpy as np

import concourse.bacc as bacc
import concourse.bass as bass
import concourse.mybir as mybir
from concourse.bass_utils import run_bass_kernel_spmd
from concourse.masks import make_identity
from concourse.tile import TileContext

H, W = 480, 640
P = 120          # partitions used (each owns R rows)
R = 4            # rows per partition
HALO = 3         # halo rows each side
CB = 4           # border cols each side (4 keeps packed reads 4B aligned)
ROWB = R + 2 * HALO          # 10 buffer rows per partition
COLB = W + 2 * CB            # 648 buffer cols
NFLAT = ROWB * COLB
RW = R * W
PROP_TIME = 8
NCORES = 8

F16 = mybir.dt.float16
F32 = mybir.dt.float32


def _make_groups():
    """9 tap groups: per (dilation, row offset), the taps whose column
    offsets form an arithmetic progression (so one DVE instruction can
    read all of them through a strided 4-dim AP).  The fused (0,0)
    center tap is a standalone 25th field: in iterations it reads the
    previous PSUM directly (f32, unshifted), so it can run while
    ScalarE is still evacuating that PSUM to the fp16 x buffer."""
    groups = []   # (d, dh, [dw...])
    slot = {}     # (dh, dw) -> (group idx, slot idx); (0,0) handled apart
    for d in (1, 2, 3):
        for r in (-1, 0, 1):
            dh = r * d
            if r == 0:
                dws = [-1, 1] if d == 1 else [-d, d]
            else:
                dws = [-d, 0, d]
            gi = len(groups)
            groups.append((d, dh, dws))
            for si, dw in enumerate(dws):
                slot[(dh, dw)] = (gi, si)
    return groups, slot


GROUPS, SLOT = _make_groups()
assert sum(len(g[2]) for g in GROUPS) == 24
# halo-independent (dh == 0) groups first -- except d3-mid, which (also
# halo-independent) goes LAST: a 2-tap group carries only 10 stop-flag
# matmul chunks instead of 15, so the next iteration's center tap (which
# waits on the stop) starts ~1us earlier; it also maximizes the slack
# between the setup-tail d3 normalization and iteration 2 reading it
_MID = [gi for gi, g in enumerate(GROUPS) if g[1] == 0]
EMIT_ORDER = [_MID[1], _MID[0]] + \
             [gi for gi, g in enumerate(GROUPS) if g[1] != 0] + [_MID[2]]


def build_nc():
    nc = bacc.Bacc("TRN2", target_bir_lowering=False, debug=False)

    g_dram = [
        nc.dram_tensor(name, [9, H, W], F32, kind="ExternalInput")
        for name in ("guided1", "guided2", "guided3")
    ]
    fuse_dram = nc.dram_tensor("fuse", [3, H, W], F32, kind="ExternalInput")
    x_dram = nc.dram_tensor("x", [1, H, W], F32, kind="ExternalInput")
    out_dram = nc.dram_tensor("out", [1, H, W], F32, kind="ExternalOutput")

    # DRAM access patterns: partition p <- rows [4p, 4p+4) (one
    # contiguous 10KB descriptor per partition)
    def rows_ap(t, extra_off=0, cols=RW):
        return bass.AP(t, extra_off, [[RW, P], [1, cols]])

    uid = [0]

    def nm(pfx):
        uid[0] += 1
        return f"{pfx}{uid[0]}"

    with TileContext(nc) as tc:
        with (
            tc.tile_pool(name="const", bufs=1) as constp,
            tc.tile_pool(name="wpool", bufs=1) as wpool,
            tc.tile_pool(name="xmain", bufs=1) as xmain,
            tc.tile_pool(name="psit", bufs=1, space="PSUM") as psi,
        ):
            ident = constp.tile([P, P], F16)
            make_identity(nc, ident)
            # shift-by-one-partition matrices: S_up moves partition p-1's
            # data to p (top halo), S_dn the reverse
            S_up = constp.tile([P, P], F16, tag="sup")
            S_dn = constp.tile([P, P], F16, tag="sdn")
            for tile_, base in ((S_up, 1), (S_dn, -1)):
                nc.gpsimd.memset(tile_, 0.0)
                nc.gpsimd.affine_select(
                    out=tile_, in_=tile_,
                    compare_op=mybir.AluOpType.not_equal,
                    fill=1.0, base=base, pattern=[[-1, P]],
                    channel_multiplier=1,
                )

            # weight group tiles (25 fields total, 5KB/partition each)
            wg = [wpool.tile([P, len(dws), R, W], F16, tag=f"wg{gi}",
                             name=f"wg{gi}")
                  for gi, (d, dh, dws) in enumerate(GROUPS)]
            wc = wpool.tile([P, 1, R, W], F16, tag="wc", name="wc")
            XA = xmain.tile([P, ROWB, COLB], F16, tag="XA")
            XB = xmain.tile([P, ROWB, COLB], F16, tag="XB")

            def slot_view(dh, dw):
                if (dh, dw) == (0, 0):
                    return wc[:, 0:1, :, :]
                gi, si = SLOT[(dh, dw)]
                return wg[gi][:, si:si + 1, :, :]       # [P, 1, R, W]

            mm_n = [0]

            def acc(ps, mflat, base, stop):
                first = mm_n[0] == 0
                for k in range(5):
                    nc.tensor.matmul(
                        out=ps[:, k * 512:(k + 1) * 512],
                        lhsT=ident,
                        rhs=mflat[:, base + k * 512:base + (k + 1) * 512],
                        start=first, stop=stop,
                    )
                    mm_n[0] += 1

            def emit_taps_single(taps, Xc, ps, pool, stop_at_end):
                """Per-tap emission (setup iteration 1, hidden under DMA)."""
                for ti, (dh, dw) in enumerate(taps):
                    m = pool.tile([P, 1, R, W], F16, tag="m0",
                                  name=nm("m0_"))
                    win = Xc[:, HALO + dh:HALO + dh + R,
                             CB + dw:CB + dw + W]
                    nc.vector.tensor_mul(
                        out=m, in0=slot_view(dh, dw), in1=win.unsqueeze(1),
                    )
                    mf = m.rearrange("p s a b -> p (s a b)")
                    acc(ps, mf, 0, stop_at_end and ti == len(taps) - 1)

            def emit_taps_batched(Xc, ps, pool3, stop_at_end,
                                  pending=None):
                """9 batched tap groups.  `pending` is the center tap's
                product: its accumulation is deferred until after the first
                group's (so the center DVE multiply is a pure READER of the
                previous PSUM and never inherits the write-after-read wait
                on the evacuation)."""
                Xfull = Xc[:, :, :]
                pstr = Xfull.ap[0][0]
                assert Xfull.ap[1][0] == COLB and Xfull.ap[2][0] == 1, \
                    Xfull.ap
                for oi, gi in enumerate(EMIT_ORDER):
                    d, dh, dws = GROUPS[gi]
                    n = len(dws)
                    step = dws[1] - dws[0]
                    m = pool3.tile([P, 3, R, W], F16, tag="m", name=nm("m_"))
                    base_off = Xfull.offset + (HALO + dh) * COLB \
                        + (CB + dws[0])
                    win = bass.AP(
                        Xfull.tensor, base_off,
                        [[pstr, P], [step, n], [COLB, R], [1, W]],
                    )
                    nc.vector.tensor_mul(
                        out=m[:, 0:n], in0=wg[gi], in1=win,
                    )
                    mf = m.rearrange("p s a b -> p (s a b)")
                    last_g = oi == len(EMIT_ORDER) - 1
                    for si in range(n):
                        acc(ps, mf, si * RW,
                            stop_at_end and last_g and si == n - 1)
                    if oi == 0 and pending is not None:
                        acc(ps, pending.rearrange("p s a b -> p (s a b)"),
                            0, False)

            HB3 = HALO * COLB      # 1944, halo bytes span per side

            def emit_tail(Xn, ps, last_iter, shiftp):
                """Evacuate PSUM, rebuild halo rows via PE partition
                shifts through single-bank PSUM chunks."""
                if last_iter:
                    return
                # the evac must NOT be high-priority: it shares the previous
                # PSUM with the center tap emitted just before it, and reader
                # serialization follows scheduler order -- the center tap has
                # to come first so it overlaps the evac
                nc.scalar.copy(
                    out=Xn[:, HALO:HALO + R, CB:CB + W],
                    in_=ps.rearrange("p (a b) -> p a b", a=R),
                )
                with tc.high_priority():
                    Xn_f = Xn.rearrange("p a b -> p (a b)")
                    for S, src0, dst0 in (
                        (S_up, (HALO + 1) * COLB, 0),
                        (S_dn, HALO * COLB, (R + HALO) * COLB),
                    ):
                        for c0 in range(0, HB3, 512):
                            ln = min(512, HB3 - c0)
                            psh = shiftp.tile([P, 512], F32, tag="sh",
                                              name=nm("sh_"))
                            nc.tensor.matmul(
                                out=psh[:, 0:ln], lhsT=S,
                                rhs=Xn_f[:, src0 + c0:src0 + c0 + ln],
                                start=True, stop=True,
                            )
                            nc.scalar.copy(
                                out=Xn_f[:, dst0 + c0:dst0 + c0 + ln],
                                in_=psh[:, 0:ln],
                            )

            # ---------------- setup + iteration 1 ----------------
            # Guided tensors stream in per dilation across 3 DMA rings;
            # iteration-1 taps of each dilation run as soon as that
            # dilation's weights are ready, hiding compute under the DMA.
            ps0 = psi.tile([P, RW], F32, tag="ps", name="ps_it0")
            with (
                tc.tile_pool(name="setup", bufs=2) as sp,
                tc.tile_pool(name="fusep", bufs=1) as fusep,
                tc.tile_pool(name="m0pool", bufs=1) as m0pool,
                tc.tile_pool(name="psst", bufs=1, space="PSUM") as psp,
            ):
                nc.vector.memset(XA, 0.0)
                nc.vector.memset(XB, 0.0)
                # three balanced DMA rings; rotated per dilation so that
                # dilation 3's last channels (the only exposed stragglers)
                # land on the faster HWDGE rings, not gpsimd's SWDGE
                engs = [nc.sync, nc.scalar, nc.gpsimd]

                def ring_of(d_idx, ch):
                    return engs[(ch + d_idx) % 3]

                for d_idx in range(2):
                    d = d_idx + 1

                    def off_of(ch):
                        return ((ch // 3 - 1) * d, (ch % 3 - 1) * d)

                    f16 = fusep.tile([P, R, W], F16, tag="f16",
                                     name=nm("f16_"))
                    e_c = fusep.tile([P, R, W], F16, tag="ec", name=nm("ec_"))
                    for ch in range(9):
                        g = sp.tile([P, RW], F32, tag="g", name=nm("g_"))
                        ring_of(d_idx, ch).dma_start(
                            out=g, in_=rows_ap(g_dram[d_idx], ch * H * W)
                        )
                        if ch == 4 and d_idx > 0:
                            dest = e_c.unsqueeze(1)
                        else:
                            dest = slot_view(*off_of(ch))
                        nc.scalar.activation(
                            out=dest.rearrange("p s a b -> p (s a b)"),
                            in_=g,
                            func=mybir.ActivationFunctionType.Exp,
                        )
                    f32t = sp.tile([P, RW], F32, tag="g", name=nm("f32t_"))
                    nc.scalar.dma_start(
                        out=f32t, in_=rows_ap(fuse_dram, d_idx * H * W)
                    )
                    nc.vector.tensor_copy(
                        out=f16, in_=f32t.rearrange("p (a b) -> p a b", a=R)
                    )
                    if d_idx == 0:
                        # x load rides behind dilation 1's channels --
                        # it's only needed once iteration-1 taps start
                        xs32 = sp.tile([P, RW], F32, tag="g")
                        nc.sync.dma_start(out=xs32, in_=rows_ap(x_dram))
                        nc.vector.tensor_copy(
                            out=XA[:, HALO:HALO + R, CB:CB + W],
                            in_=xs32.rearrange("p (a b) -> p a b", a=R),
                        )
                        XA_f = XA.rearrange("p a b -> p (a b)")
                        nc.sync.dma_start(
                            out=XA_f[1:P, 0:HALO * COLB],
                            in_=XA_f[0:P - 1, R * COLB:(R + HALO) * COLB],
                        )
                        nc.sync.dma_start(
                            out=XA_f[0:P - 1, (R + HALO) * COLB:NFLAT],
                            in_=XA_f[1:P, HALO * COLB:2 * HALO * COLB],
                        )
                    # channel sums + 1/sum in halves (3 PSUM banks)
                    t_ = fusep.tile([P, RW], F16, tag="t", name=nm("t_"))
                    f16f = f16.rearrange("p a b -> p (a b)")
                    HB = RW // 2
                    for h0 in (0, HB):
                        pss = psp.tile([P, HB], F32, tag="pss",
                                       name=nm("pss_"))
                        for ch in range(9):
                            if ch == 4 and d_idx > 0:
                                sf = e_c.rearrange("p a b -> p (a b)")
                            else:
                                sf = slot_view(*off_of(ch)).rearrange(
                                    "p s a b -> p (s a b)")
                            for c0 in range(0, HB, 512):
                                c1 = min(c0 + 512, HB)
                                nc.tensor.matmul(
                                    out=pss[:, c0:c1], lhsT=ident,
                                    rhs=sf[:, h0 + c0:h0 + c1],
                                    start=(ch == 0), stop=(ch == 8),
                                )
                        r = fusep.tile([P, HB], F32, tag="r", name=nm("r_"))
                        nc.vector.reciprocal_approx_fast(out=r, in_=pss)
                        nc.vector.tensor_mul(
                            out=t_[:, h0:h0 + HB], in0=f16f[:, h0:h0 + HB],
                            in1=r,
                        )
                    tv = t_.rearrange("p (a b) -> p a b", a=R)
                    # normalize this dilation's groups in-place
                    for gi in (3 * d_idx, 3 * d_idx + 1, 3 * d_idx + 2):
                        n = len(GROUPS[gi][2])
                        tvb = tv.unsqueeze(1).broadcast_to((P, n, R, W))
                        nc.vector.tensor_mul(out=wg[gi], in0=wg[gi], in1=tvb)
                    if d_idx == 0:
                        # center field: exp(g1_c) -> k1_c * f1
                        nc.vector.tensor_mul(
                            out=wc, in0=wc, in1=tv.unsqueeze(1),
                        )
                    else:
                        cview = slot_view(0, 0)
                        nc.vector.tensor_mul(
                            out=e_c, in0=e_c, in1=tv,
                        )
                        nc.vector.tensor_add(
                            out=cview, in0=cview, in1=e_c.unsqueeze(1),
                        )
                    # iteration-1 taps of this dilation; the shared center
                    # field holds d1+d2's contribution once d2 is folded in,
                    # so the center tap runs at the end of d2 (d3's center
                    # share rides in the raw-tap path below)
                    taps = [off_of(ch) for ch in range(9) if ch != 4]
                    if d_idx == 1:
                        taps.append((0, 0))
                    emit_taps_single(taps, XA, ps0, m0pool, stop_at_end=False)

                # ---- dilation 3: iteration-1 via RAW exp taps ----
                # d3's weights land last, so anything serialized after them
                # is exposed.  Instead of waiting for the softmax
                # normalization, accumulate sum_ch exp(g_ch) * shift(x) in
                # fp16 SBUF and the channel sum on DVE as each channel
                # arrives (all hidden under the DMA stream), then apply the
                # per-pixel fuse3/sum3 scale once at the end.
                f16 = fusep.tile([P, R, W], F16, tag="f16", name=nm("f16_"))
                e_c = fusep.tile([P, R, W], F16, tag="ec", name=nm("ec_"))
                raw = fusep.tile([P, 1, R, W], F16, tag="raw")
                sacc = fusep.tile([P, 1, R, W], F16, tag="sacc")
                f32t = sp.tile([P, RW], F32, tag="g", name=nm("f32t3_"))
                nc.gpsimd.dma_start(
                    out=f32t, in_=rows_ap(fuse_dram, 2 * H * W)
                )
                nc.vector.tensor_copy(
                    out=f16, in_=f32t.rearrange("p (a b) -> p a b", a=R)
                )
                for ch in range(9):
                    g = sp.tile([P, RW], F32, tag="g", name=nm("g_"))
                    ring_of(2, ch).dma_start(
                        out=g, in_=rows_ap(g_dram[2], ch * H * W)
                    )
                    dh, dw = (ch // 3 - 1) * 3, (ch % 3 - 1) * 3
                    if ch == 4:
                        dest = e_c.unsqueeze(1)
                    else:
                        dest = slot_view(dh, dw)
                    nc.scalar.activation(
                        out=dest.rearrange("p s a b -> p (s a b)"), in_=g,
                        func=mybir.ActivationFunctionType.Exp,
                    )
                    win = XA[:, HALO + dh:HALO + dh + R,
                             CB + dw:CB + dw + W].unsqueeze(1)
                    if ch == 0:
                        nc.vector.tensor_mul(out=raw, in0=dest, in1=win)
                        nc.vector.tensor_copy(out=sacc, in_=dest)
                    else:
                        pr = m0pool.tile([P, 1, R, W], F16, tag="m0",
                                         name=nm("pr_"))
                        nc.vector.tensor_mul(out=pr, in0=dest, in1=win)
                        nc.vector.tensor_add(out=raw, in0=raw, in1=pr)
                        nc.vector.tensor_add(out=sacc, in0=sacc, in1=dest)
                # t3 = fuse3 / sum3 (per pixel); the f32 staging borrows a
                # free g-buffer (the channel stream is over), reciprocal in
                # halves to reuse the d1/d2-sized tiles
                s32 = sp.tile([P, RW], F32, tag="g", name=nm("s32_"))
                nc.vector.tensor_copy(
                    out=s32, in_=sacc.rearrange("p s a b -> p (s a b)")
                )
                t_ = fusep.tile([P, RW], F16, tag="t", name=nm("t3_"))
                saccf = s32
                f16f3 = f16.rearrange("p a b -> p (a b)")
                HB = RW // 2
                for h0 in (0, HB):
                    rh = fusep.tile([P, HB], F32, tag="r", name=nm("r3_"))
                    nc.vector.reciprocal_approx_fast(
                        out=rh, in_=saccf[:, h0:h0 + HB]
                    )
                    nc.vector.tensor_mul(
                        out=t_[:, h0:h0 + HB], in0=f16f3[:, h0:h0 + HB],
                        in1=rh,
                    )
                tv = t_.rearrange("p (a b) -> p a b", a=R)
                # iteration-1 d3 contribution (carries the stop flags)
                m3 = m0pool.tile([P, 1, R, W], F16, tag="m0", name=nm("m3_"))
                nc.vector.tensor_mul(out=m3, in0=raw, in1=tv.unsqueeze(1))
                acc(ps0, m3.rearrange("p s a b -> p (s a b)"), 0, True)
                assert mm_n[0] == 90
                # weights for iterations 2..8: fold e_c into the shared
                # center field and normalize d3's groups (these run on DVE
                # concurrently with iteration 1's PSUM evacuation)
                cview = slot_view(0, 0)
                nc.vector.tensor_mul(out=e_c, in0=e_c, in1=tv)
                nc.vector.tensor_add(
                    out=cview, in0=cview, in1=e_c.unsqueeze(1)
                )
                for gi in (6, 7, 8):
                    n = len(GROUPS[gi][2])
                    tvb = tv.unsqueeze(1).broadcast_to((P, n, R, W))
                    nc.vector.tensor_mul(out=wg[gi], in0=wg[gi], in1=tvb)

            # ---------------- iterations 2..8 ----------------
            with tc.tile_pool(name="shp", bufs=3, space="PSUM") as shiftp:
                with (
                    tc.tile_pool(name="mpool", bufs=3) as mpool,
                    tc.tile_pool(name="mcpool", bufs=1) as mcpool,
                ):
                    bufs = [XA, XB]
                    ps = ps0
                    for it in range(1, PROP_TIME):
                        Xc = bufs[it % 2]
                        mm_n[0] = 0
                        new_ps = psi.tile([P, RW], F32, tag="ps",
                                          name=nm("ps_"))
                        # center tap: FIRST reader of the previous PSUM,
                        # emitted before its evacuation so DVE runs it while
                        # ScalarE evacuates (readers are serialized in
                        # emission order)
                        mC = mcpool.tile([P, 1, R, W], F16, tag="mc",
                                         name=nm("mc_"))
                        pswin = ps.rearrange("p (a b) -> p a b",
                                             a=R).unsqueeze(1)
                        nc.vector.tensor_mul(out=mC, in0=wc, in1=pswin)
                        emit_tail(Xc, ps, False, shiftp)
                        emit_taps_batched(Xc, new_ps, mpool,
                                          stop_at_end=True, pending=mC)
                        assert mm_n[0] == 125
                        ps = new_ps

                # final iteration: evacuate PSUM in halves so the DMA of
                # the first half overlaps the copy of the second, and the
                # two DMAs ride different HWDGE rings
                with tc.tile_pool(name="stagep", bufs=1) as stagep:
                    stage = stagep.tile([P, RW], F32)
                    nc.scalar.copy(out=stage[:, 0:RW // 2],
                                   in_=ps[:, 0:RW // 2])
                    nc.sync.dma_start(
                        out=rows_ap(out_dram, 0, cols=RW // 2),
                        in_=stage[:, 0:RW // 2],
                    )
                    nc.scalar.copy(out=stage[:, RW // 2:RW],
                                   in_=ps[:, RW // 2:RW])
                    nc.scalar.dma_start(
                        out=rows_ap(out_dram, RW // 2, cols=RW // 2),
                        in_=stage[:, RW // 2:RW],
                    )

    nc.compile()
    return nc


_NC = None


def _get_nc():
    global _NC
    if _NC is None:
        _NC = build_nc()
    return _NC


def _in_maps(guided1, guided2, guided3, fuse, x):
    maps = []
    for b in range(NCORES):
        maps.append({
            "guided1": np.ascontiguousarray(guided1[b], dtype=np.float32),
            "guided2": np.ascontiguousarray(guided2[b], dtype=np.float32),
            "guided3": np.ascontiguousarray(guided3[b], dtype=np.float32),
            "fuse": np.ascontiguousarray(fuse[b], dtype=np.float32),
            "x": np.ascontiguousarray(x[b], dtype=np.float32),
        })
    return maps


def kernel(guided1, guided2, guided3, fuse, x):
    nc = _get_nc()
    res = run_bass_kernel_spmd(
        nc, _in_maps(guided1, guided2, guided3, fuse, x),
        core_ids=list(range(NCORES)),
    )
    return np.stack([res.results[b]["out"] for b in range(NCORES)], axis=0)


def kernel_profiled(guided1, guided2, guided3, fuse, x, tmpdir=None):
    """Returns (output, BassKernelResults) with trace enabled."""
    nc = _get_nc()
    res = run_bass_kernel_spmd(
        nc, _in_maps(guided1, guided2, guided3, fuse, x),
        core_ids=list(range(NCORES)), trace=True, tmpdir=tmpdir,
    )
    out = np.stack([res.results[b]["out"] for b in range(NCORES)], axis=0)
    return out, res
